# revision 2
# baseline (speedup 1.0000x reference)
"""DiT attention block on 8 Trainium2 NeuronCores.

Sharding: batch (2) x head-groups (4 heads each) -> 8 cores.
Each core computes, for its batch b and 4 heads:
    q/k/v projections, RMSNorm+rope on q/k, softmax attention, and its
    partial output projection out_partial^T = Wo_slice^T-contraction.
Host sums the 4 head-group partials per batch and transposes back.

Layouts on device (chosen so every matmul has its contraction dim on
partitions and softmax denominators come out of cheap PE reductions):
    xT      [D, S]   = x[b].T
    wqT/wkT [D, 512] = Wq/Wk row-slice transposed (matmul lhsT layout)
    wvT     [D, 512] = Wv row-slice transposed (matmul rhs layout)
    woT     [512, D] = Wo column-slice transposed (lhsT tiles)
    q/k produced transposed per head: [d=128, S]; v in [S, 512].
    scores computed transposed ([k, q]) so exp(scores) feeds the A@V
    matmul directly as the moving operand.
"""

import math

import numpy as np

import concourse.bass as bass
import concourse.mybir as mybir
import concourse.tile as tile
from concourse.bass_utils import run_bass_kernel_spmd

F32 = mybir.dt.float32
P = 128          # partitions / head_dim
S = 2048         # sequence
D = 2048         # model dim
HD = 128         # head dim
NH = 16          # total heads
NHL = 4          # heads per core
IL = NHL * HD    # 512, inner slice per core
KO = D // P      # 16 contraction tiles
SC = 256         # x-chunk columns in the QKV phase
NSC = S // SC    # 8
QC = 512         # q-chunk columns in attention / out phases
NQC = S // QC    # 4
EPS = 1e-6
SCALE = 1.0 / math.sqrt(HD)
N_CORES = 8

_PROG_CACHE = {}


def _split_multi_waits(nc, max_waits=1):
    """walrus here rejects >1 sync-wait per instruction; move extras onto
    same-engine nops placed immediately before the instruction."""
    n_split = 0
    for fn in nc.m.functions:
        for bb in fn.blocks:
            insts = bb.instructions
            new_list = []
            changed = False
            for inst in insts:
                si = getattr(inst, "sync_info", None)
                waits = list(si.on_wait) if (si is not None and si.on_wait) else []
                if len(waits) > max_waits:
                    extra = waits[:-max_waits]
                    keep = waits[-max_waits:]
                    for i in range(0, len(extra), max_waits):
                        nop = mybir.InstNoOp(
                            name=f"I-wsplit-{nc.next_id()}", ins=[], outs=[])
                        nop.engine = inst.engine
                        nop.sync_info = mybir.SyncInfo(
                            on_wait=extra[i:i + max_waits], on_update=[])
                        new_list.append(nop)
                        n_split += 1
                    del si.on_wait[:]
                    si.on_wait.extend(keep)
                    changed = True
                new_list.append(inst)
            if changed:
                del insts[:]
                insts.extend(new_list)
    return n_split


def _emit(nc, tc, t):
    from contextlib import ExitStack

    Act = mybir.ActivationFunctionType

    with ExitStack() as top:
        const = top.enter_context(tc.tile_pool(name="const", bufs=1))

        rope = {}
        for nm in ("ropeAq", "ropeBq", "ropeAk", "ropeBk"):
            til = const.tile([P, S], F32, tag=nm)
            nc.sync.dma_start(til[:], t[nm][:, :])
            rope[nm] = til
        pswap = const.tile([P, P], F32, tag="pswap")
        nc.sync.dma_start(pswap[:], t["pswap"][:, :])
        ones_col = const.tile([P, 1], F32, tag="ones_col")
        nc.vector.memset(ones_col, 1.0)
        ones_row = const.tile([1, P], F32, tag="ones_row")
        nc.vector.memset(ones_row, 1.0)
        eps_t = const.tile([1, 1], F32, tag="eps")
        nc.vector.memset(eps_t, EPS)

        dramp = top.enter_context(tc.tile_pool(name="spill", bufs=1, space="DRAM"))
        qnT = dramp.tile([IL, S], F32, tag="qnT")
        knT = dramp.tile([IL, S], F32, tag="knT")
        vsd = dramp.tile([S, IL], F32, tag="vsd")

        # ---------------- phase 1: QKV projections + RMSNorm + rope ---------
        with ExitStack() as ph:
            wpool = ph.enter_context(tc.tile_pool(name="w", bufs=1))
            xpool = ph.enter_context(tc.tile_pool(name="x", bufs=2))
            tp = ph.enter_context(tc.tile_pool(name="qkvtmp", bufs=3))
            ps = ph.enter_context(tc.tile_pool(name="ps_qkv", bufs=4, space="PSUM"))
            ps1 = ph.enter_context(tc.tile_pool(name="ps1_qkv", bufs=2, space="PSUM"))

            wq = wpool.tile([P, KO, IL], F32, tag="wq")
            nc.sync.dma_start(wq[:], t["wqT"][:, :].rearrange("(ko p) i -> p ko i", p=P))
            wk = wpool.tile([P, KO, IL], F32, tag="wk")
            nc.sync.dma_start(wk[:], t["wkT"][:, :].rearrange("(ko p) i -> p ko i", p=P))
            wv = wpool.tile([P, KO, IL], F32, tag="wv")
            nc.sync.dma_start(wv[:], t["wvT"][:, :].rearrange("(ko p) i -> p ko i", p=P))

            xT_r = t["xT"][:, :].rearrange("(ko p) s -> p ko s", p=P)

            for sc in range(NSC):
                xt = xpool.tile([P, KO, SC], F32, tag="xchunk")
                nc.sync.dma_start(xt[:], xT_r[:, :, sc * SC:(sc + 1) * SC])

                # V projection ([s, i] layout, lhsT = x subtiles)
                for st in range(SC // P):
                    pv = ps.tile([P, QC], F32, tag="ps_main")
                    for kk in range(KO):
                        nc.tensor.matmul(
                            pv[:, :],
                            lhsT=xt[:, kk, st * P:(st + 1) * P],
                            rhs=wv[:, kk, :],
                            start=(kk == 0), stop=(kk == KO - 1))
                    vchunk = tp.tile([P, IL], F32, tag="vchunk")
                    nc.scalar.copy(vchunk[:], pv[:, :])
                    row0 = sc * SC + st * P
                    nc.sync.dma_start(vsd[row0:row0 + P, :], vchunk[:])

                # Q / K projections in transposed layout + norm + rope
                for wt, ra, rb, dst in (
                    (wq, rope["ropeAq"], rope["ropeBq"], qnT),
                    (wk, rope["ropeAk"], rope["ropeBk"], knT),
                ):
                    for h in range(NHL):
                        pqk = ps.tile([P, QC], F32, tag="ps_main")
                        for kk in range(KO):
                            nc.tensor.matmul(
                                pqk[:, :SC],
                                lhsT=wt[:, kk, h * P:(h + 1) * P],
                                rhs=xt[:, kk, :],
                                start=(kk == 0), stop=(kk == KO - 1))
                        raw = tp.tile([P, SC], F32, tag="raw")
                        nc.scalar.copy(raw[:], pqk[:, :SC])
                        # sum of squares over head_dim (partitions) on PE
                        sq = tp.tile([P, SC], F32, tag="sq")
                        nc.vector.tensor_mul(sq[:], raw[:], raw[:])
                        pssq = ps1.tile([1, QC], F32, tag="ps_ssq")
                        nc.tensor.matmul(pssq[:, :SC], lhsT=ones_col[:], rhs=sq[:],
                                         start=True, stop=True)
                        srt = tp.tile([1, SC], F32, tag="srt")
                        nc.scalar.activation(srt[:], pssq[:, :SC], func=Act.Sqrt,
                                             bias=eps_t[:], scale=1.0 / HD)
                        rstd = tp.tile([1, SC], F32, tag="rstd")
                        nc.vector.reciprocal(rstd[:], srt[:])
                        # rope: rotate-half swap via permutation matmul
                        psw = ps.tile([P, QC], F32, tag="ps_main")
                        nc.tensor.matmul(psw[:, :SC], lhsT=pswap[:], rhs=raw[:],
                                         start=True, stop=True)
                        tmp = tp.tile([P, SC], F32, tag="ropetmp")
                        nc.vector.tensor_mul(
                            tmp[:], ra[:, sc * SC:(sc + 1) * SC], raw[:])
                        tmp2 = tp.tile([P, SC], F32, tag="ropetmp2")
                        nc.vector.tensor_mul(
                            tmp2[:], rb[:, sc * SC:(sc + 1) * SC], psw[:, :SC])
                        nc.vector.tensor_add(tmp[:], tmp[:], tmp2[:])
                        # apply rstd (broadcast over partitions via K=1 matmul)
                        pb = ps.tile([P, QC], F32, tag="ps_main")
                        nc.tensor.matmul(pb[:, :SC], lhsT=ones_row[:], rhs=rstd[:],
                                         start=True, stop=True)
                        qn = tp.tile([P, SC], F32, tag="qn")
                        nc.vector.tensor_mul(qn[:], tmp[:], pb[:, :SC])
                        nc.sync.dma_start(
                            dst[h * P:(h + 1) * P, sc * SC:(sc + 1) * SC], qn[:])

        # ---------------- phase 2+3: attention, then output projection ------
        with ExitStack() as ph:
            avnp = ph.enter_context(tc.tile_pool(name="avn", bufs=1))
            avn = [avnp.tile([P, S], F32, tag=f"avn{h}", name=f"avn{h}")
                   for h in range(NHL)]

            with ExitStack() as ap_:
                qpool = ap_.enter_context(tc.tile_pool(name="attq", bufs=2))
                kpool = ap_.enter_context(tc.tile_pool(name="attk", bufs=2))
                vpool = ap_.enter_context(tc.tile_pool(name="attv", bufs=2))
                apool = ap_.enter_context(tc.tile_pool(name="attnT", bufs=3))
                accp = ap_.enter_context(tc.tile_pool(name="acc", bufs=2))
                smt = ap_.enter_context(tc.tile_pool(name="smallt", bufs=2))
                ps_av = ap_.enter_context(
                    tc.tile_pool(name="ps_av", bufs=4, space="PSUM"))
                ps_sc = ap_.enter_context(
                    tc.tile_pool(name="ps_sc", bufs=3, space="PSUM"))
                ps_rs = ap_.enter_context(
                    tc.tile_pool(name="ps_rs", bufs=1, space="PSUM"))

                vsd_r = vsd[:, :].rearrange("(ko p) i -> p ko i", p=P)

                for h in range(NHL):
                    qn_h = qpool.tile([P, S], F32, tag="qn_h")
                    nc.sync.dma_start(qn_h[:], qnT[h * P:(h + 1) * P, :])
                    kn_h = kpool.tile([P, S], F32, tag="kn_h")
                    nc.sync.dma_start(kn_h[:], knT[h * P:(h + 1) * P, :])
                    v_h = vpool.tile([P, KO, HD], F32, tag="v_h")
                    nc.sync.dma_start(v_h[:], vsd_r[:, :, h * HD:(h + 1) * HD])

                    pav = [ps_av.tile([P, QC], F32, tag="ps_avt", name=f"pav{i}")
                           for i in range(NQC)]
                    acc = accp.tile([P, S], F32, tag="acc")

                    for kt in range(KO):
                        at = apool.tile([P, S], F32, tag="at")
                        for qc in range(NQC):
                            psc = ps_sc.tile([P, QC], F32, tag="ps_sct")
                            nc.tensor.matmul(
                                psc[:],
                                lhsT=kn_h[:, kt * P:(kt + 1) * P],
                                rhs=qn_h[:, qc * QC:(qc + 1) * QC],
                                start=True, stop=True)
                            nc.scalar.activation(
                                at[:, qc * QC:(qc + 1) * QC], psc[:],
                                func=Act.Exp, scale=SCALE)
                            nc.tensor.matmul(
                                pav[qc][:],
                                lhsT=v_h[:, kt, :],
                                rhs=at[:, qc * QC:(qc + 1) * QC],
                                start=(kt == 0), stop=(kt == KO - 1))
                        if kt == 0:
                            nc.vector.tensor_copy(acc[:], at[:])
                        else:
                            nc.vector.tensor_add(acc[:], acc[:], at[:])

                    # softmax denominator: partition-sum of acc, recip, bcast
                    for qc in range(NQC):
                        prs = ps_rs.tile([1, QC], F32, tag="ps_rst")
                        nc.tensor.matmul(prs[:], lhsT=ones_col[:],
                                         rhs=acc[:, qc * QC:(qc + 1) * QC],
                                         start=True, stop=True)
                        rs = smt.tile([1, QC], F32, tag="rs")
                        nc.vector.reciprocal(rs[:], prs[:])
                        prb = ps_sc.tile([P, QC], F32, tag="ps_sct")
                        nc.tensor.matmul(prb[:], lhsT=ones_row[:], rhs=rs[:],
                                         start=True, stop=True)
                        rbs = smt.tile([P, QC], F32, tag="rbs")
                        nc.scalar.copy(rbs[:], prb[:])
                        nc.vector.tensor_mul(
                            avn[h][:, qc * QC:(qc + 1) * QC], pav[qc][:], rbs[:])

            # ---------------- output projection -----------------------------
            with ExitStack() as op_:
                wop = op_.enter_context(tc.tile_pool(name="wo", bufs=3))
                otp = op_.enter_context(tc.tile_pool(name="ot", bufs=4))
                ps_o = op_.enter_context(
                    tc.tile_pool(name="ps_o", bufs=8, space="PSUM"))

                woT_r = t["woT"][:, :].rearrange("(it p) d -> p it d", p=P)
                for dt in range(D // P):
                    wo_t = wop.tile([P, NHL, P], F32, tag="wo_t")
                    nc.sync.dma_start(wo_t[:], woT_r[:, :, dt * P:(dt + 1) * P])
                    po = [ps_o.tile([P, QC], F32, tag="ps_ot", name=f"po{i}")
                          for i in range(NQC)]
                    for it in range(NHL):
                        for qc in range(NQC):
                            nc.tensor.matmul(
                                po[qc][:],
                                lhsT=wo_t[:, it, :],
                                rhs=avn[it][:, qc * QC:(qc + 1) * QC],
                                start=(it == 0), stop=(it == NHL - 1))
                    for qc in range(NQC):
                        ot = otp.tile([P, QC], F32, tag="ot")
                        nc.scalar.copy(ot[:], po[qc][:])
                        nc.sync.dma_start(
                            t["outT"][dt * P:(dt + 1) * P,
                                      qc * QC:(qc + 1) * QC], ot[:])


def _build_program():
    if "nc" in _PROG_CACHE:
        return _PROG_CACHE["nc"]
    nc = bass.Bass()
    t = {}
    t["xT"] = nc.dram_tensor("xT", [D, S], F32, kind="ExternalInput")
    t["wqT"] = nc.dram_tensor("wqT", [D, IL], F32, kind="ExternalInput")
    t["wkT"] = nc.dram_tensor("wkT", [D, IL], F32, kind="ExternalInput")
    t["wvT"] = nc.dram_tensor("wvT", [D, IL], F32, kind="ExternalInput")
    t["woT"] = nc.dram_tensor("woT", [IL, D], F32, kind="ExternalInput")
    for nm in ("ropeAq", "ropeBq", "ropeAk", "ropeBk"):
        t[nm] = nc.dram_tensor(nm, [P, S], F32, kind="ExternalInput")
    t["pswap"] = nc.dram_tensor("pswap", [P, P], F32, kind="ExternalInput")
    t["outT"] = nc.dram_tensor("outT", [D, S], F32, kind="ExternalOutput")

    with tile.TileContext(nc) as tc:
        _emit(nc, tc, t)
    _split_multi_waits(nc)
    _PROG_CACHE["nc"] = nc
    return nc


def _prep_in_maps(x, rope_emb, Wq, Wk, Wv, Wo, q_norm_w, k_norm_w):
    x = np.asarray(x, np.float32)
    F = np.asarray(rope_emb, np.float32)[:, 0]          # [S, 64, 2, 2]
    A0 = np.concatenate([F[:, :, 0, 0], F[:, :, 1, 1]], axis=-1)  # [S, 128]
    B0 = np.concatenate([F[:, :, 0, 1], F[:, :, 1, 0]], axis=-1)  # [S, 128]

    def rope_consts(w):
        w = np.asarray(w, np.float32)
        w_sw = np.concatenate([w[64:], w[:64]])
        A = np.ascontiguousarray((A0 * w[None, :]).T)    # [128, S]
        B = np.ascontiguousarray((B0 * w_sw[None, :]).T)
        return A, B

    Aq, Bq = rope_consts(q_norm_w)
    Ak, Bk = rope_consts(k_norm_w)
    pswap = np.zeros((P, P), np.float32)
    for d in range(P):
        pswap[(d + 64) % P, d] = 1.0

    xT = [np.ascontiguousarray(x[b].T) for b in range(x.shape[0])]
    Wq = np.asarray(Wq, np.float32)
    Wk = np.asarray(Wk, np.float32)
    Wv = np.asarray(Wv, np.float32)
    Wo = np.asarray(Wo, np.float32)

    in_maps = []
    for c in range(N_CORES):
        b, hg = divmod(c, NH // NHL)
        sl = slice(hg * IL, (hg + 1) * IL)
        in_maps.append({
            "xT": xT[b],
            "wqT": np.ascontiguousarray(Wq[sl, :].T),
            "wkT": np.ascontiguousarray(Wk[sl, :].T),
            "wvT": np.ascontiguousarray(Wv[sl, :].T),
            "woT": np.ascontiguousarray(Wo[:, sl].T),
            "ropeAq": Aq, "ropeBq": Bq, "ropeAk": Ak, "ropeBk": Bk,
            "pswap": pswap,
        })
    return in_maps


def kernel(x, rope_emb, Wq, Wk, Wv, Wo, q_norm_w, k_norm_w, _trace=False):
    nc = _build_program()
    in_maps = _prep_in_maps(x, rope_emb, Wq, Wk, Wv, Wo, q_norm_w, k_norm_w)
    res = run_bass_kernel_spmd(nc, in_maps, core_ids=list(range(N_CORES)),
                               trace=_trace)
    out = np.empty((2, S, D), np.float32)
    for b in range(2):
        acc = res.results[4 * b]["outT"].copy()
        for hg in range(1, 4):
            acc += res.results[4 * b + hg]["outT"]
        out[b] = acc.T
    if _trace:
        kernel.last_exec_time_ns = res.exec_time_ns
        kernel.last_results = res
    return out
